# revision 43
# baseline (speedup 1.0000x reference)
"""Multi-head attention (B=2, S=2048, D=1024, H=16) on 8 Trainium2 cores.

Sharding: core c handles batch b = c//4 and head group g = c%4 (4 heads).
Output projection is row-sharded over head dims; per-core partial outputs
are summed on the host (bias added on the host).

Design (ACT-bound): the scalar engine's exp is the hard floor
(131072 cols/core @ 1.2GHz + ~300cyc/instr overhead ~= 142us), so the whole
schedule exists to keep ACT 100% busy on uniform [128,1024] exp tiles:

  - psS has THREE [128,1024] f32 slots (6 PSUM banks). With two QK tiles
    allocated per step, the 3-slot rotation opens each QK's WAR gate a full
    exp (~1.1us) before the pipeline needs it, so the two heads' K=64
    matmuls run as a concurrent row-group pair (full-array MAC activity)
    and ACT never waits on the in-order PE queue.
  - HAM (the PE clock gate) re-throttles unless the MAC array is nearly
    saturated per 3.4us window; garbage "fill" matmuls (K=128, written into
    the previous step's already-read S tile) top every step up to ~2.1us of
    PE time. On steps with little gated work the fills are capped: their
    write target's exp is still in flight when the PE runs ahead, so extra
    fills behind that gate would only delay the next step's QK.
  - PV: prev-block head1 on steps 3..8, own head0 on steps 10..15; the last
    j-block + norm run at the NEXT block's step 1, after their gating exp
    has retired (V is augmented with a ones column so PSUM row 0 is the
    softmax denominator; start offsets clear the norm chain's psO slot).
  - Tail: block 3's head-1 PV accumulates in a freed psS slot, then the
    whole output projection pipelines through the remaining psS slots
    ([128,1024] Y tiles, copies alternating DVE/ACT, y output in bf16).
norm: DVE reciprocal + gpsimd partition-broadcast + DVE multiply; head1
writes at_sb in place (rows 64-127), head0 stages + SBUF DMA (row shift).
"""

import sys

sys.path.insert(0, "/opt/trn_rl_repo")

from contextlib import ExitStack

import numpy as np
import ml_dtypes

import concourse.bass as bass
import concourse.tile as tile
from concourse import bacc, mybir

N_CORES = 8
B, S, D_MODEL = 2, 2048, 1024
NUM_HEADS, D_K = 16, 64
H_PER_CORE = 4
SCALE = D_K ** -0.5
IS = 1024                 # i-super width
JB = S // 128             # 16 j-blocks
VA_W = 128                # ones col 0, zeros 1-63, v at 64-127
VA_CHUNK = 4              # va split into 4 tiles of 4 j-blocks each
ET_BUFS = 44

F32 = mybir.dt.float32
BF16 = mybir.dt.bfloat16
AF = mybir.ActivationFunctionType
BLOCKS = [(0, 0), (1, 0), (0, 1024), (1, 1024)]  # (pair, i0), iw = 1024

PE_TARGET = 2160          # ns of PE time to emit per step (ACT step ~2250)
QK_PE = 713               # measured QK quad wall (LDW stagger included)
PV_PE = 426               # per j-block (2 chunk matmuls)
FILL_MAC = 213

# per-step jbp batches: prev-block head1 over steps 3..8 (start clears the
# preceding norm chain's psO occupancy), own head0 over steps 10..15; the
# last j-block + norm run at the NEXT block's step 1, after its gating exp
# has certainly retired, so the in-order PE queue never stalls on them
PV1_SCHED = {3: [0, 1, 2], 4: [3, 4, 5], 5: [6, 7, 8], 6: [9, 10, 11],
             7: [12, 13, 14, 15]}
PV0_SCHED = {10: [0, 1, 2], 11: [3, 4, 5], 12: [6, 7, 8], 13: [9, 10, 11],
             14: [12, 13], 15: [14]}


def ds(start, size):
    return slice(start, start + size)


def _trace(ctx: ExitStack, tc: tile.TileContext, io: dict):
    nc = tc.nc

    const = ctx.enter_context(tc.tile_pool(name="const", bufs=1))
    etp = ctx.enter_context(tc.tile_pool(name="et", bufs=ET_BUFS))
    normp = ctx.enter_context(tc.tile_pool(name="norm", bufs=2))
    atp = ctx.enter_context(tc.tile_pool(name="at", bufs=1))
    youtp = ctx.enter_context(tc.tile_pool(name="yout", bufs=6))
    miscp = ctx.enter_context(tc.tile_pool(name="misc", bufs=2))
    psS = ctx.enter_context(tc.tile_pool(name="psS", bufs=3, space="PSUM"))
    psO = ctx.enter_context(tc.tile_pool(name="psO", bufs=1, space="PSUM"))

    # ---- resident inputs (order matters: earliest-needed first) ----
    kt_sb = [const.tile([128, S], BF16, tag=f"kt{p}", name=f"kt{p}")
             for p in range(2)]
    qt_sb = [const.tile([128, S], BF16, tag=f"qt{p}", name=f"qt{p}")
             for p in range(2)]
    va_sb = [const.tile([128, JB // VA_CHUNK * H_PER_CORE * VA_W], BF16,
                        tag=f"va{c}", name=f"va{c}") for c in range(VA_CHUNK)]
    nc.sync.dma_start(kt_sb[0][:, 0:128], io["kt"][0][:, 0:128])
    nc.sync.dma_start(qt_sb[0][:, 0:IS], io["qt"][0][:, 0:IS])
    nc.sync.dma_start(kt_sb[0][:, 128:512], io["kt"][0][:, 128:512])
    nc.sync.dma_start(va_sb[0][:], io["va"][0])
    nc.sync.dma_start(kt_sb[0][:, 512:S], io["kt"][0][:, 512:S])
    nc.sync.dma_start(va_sb[1][:], io["va"][1])
    nc.sync.dma_start(kt_sb[1][:], io["kt"][1])
    nc.sync.dma_start(qt_sb[1][:, 0:IS], io["qt"][1][:, 0:IS])
    nc.sync.dma_start(va_sb[2][:], io["va"][2])
    nc.sync.dma_start(va_sb[3][:], io["va"][3])
    wt_sb = []
    for p in range(2):
        t = const.tile([128, D_MODEL], BF16, tag=f"wt{p}")
        nc.sync.dma_start(t[:], io["wt"][p])
        wt_sb.append(t)
    nc.sync.dma_start(qt_sb[0][:, IS:S], io["qt"][0][:, IS:S])
    nc.sync.dma_start(qt_sb[1][:, IS:S], io["qt"][1][:, IS:S])
    at_sb = [atp.tile([128, S], BF16, tag=f"at{p}", name=f"at{p}")
             for p in range(2)]

    # ---- warmup ----
    # exp-table preload on ACT (reads the first kt sliver, so the ~2.7us
    # table load overlaps input DMA), then 4 full-array matmuls for HAM
    wexp = miscp.tile([1, 128], F32, tag="warm_exp", name="warm_exp")
    nc.scalar.activation(wexp[:], kt_sb[0][0:1, 0:128], AF.Exp, scale=SCALE)
    nc.sync.dma_start(io["wexp"][:], wexp[:])
    wps = psS.tile([128, IS], F32, tag="S", name="warm_ps")
    nc.tensor.matmul(wps[:, 0:128], kt_sb[0][:, 0:128], kt_sb[0][:, 0:128],
                     start=True, stop=True, skip_group_check=True)
    wsb = miscp.tile([1, 128], F32, tag="warm_out", name="warm_out")
    nc.vector.tensor_copy(wsb[:], wps[0:1, 0:128])
    nc.sync.dma_start(io["warm"][:], wsb[:])

    ET = {}    # (block_idx, h2) -> list of 16 E tiles
    PSO = {}   # (block_idx, h2) -> psum tile
    dead_sp = [wps]  # S tiles whose exp already ran: fill targets

    def emit_qk_exp(t, jb):
        pr, i0 = BLOCKS[t]
        # interleave the two heads' matmuls: distinct PE row-groups run
        # concurrently (full-array MAC activity keeps HAM at 8/8)
        sps = [psS.tile([128, IS], F32, tag="S", name="sp") for _ in range(2)]
        for nch in range(2):
            for h2 in range(2):
                nc.tensor.matmul(
                    sps[h2][:, ds(nch * 512, 512)],
                    kt_sb[pr][ds(h2 * 64, 64), ds(jb * 128, 128)],
                    qt_sb[pr][ds(h2 * 64, 64), ds(i0 + nch * 512, 512)],
                    start=True, stop=True,
                )
        for h2 in range(2):
            e = etp.tile([128, IS], BF16, tag="et", name="e")
            nc.scalar.activation(e[:], sps[h2][:], AF.Exp, scale=SCALE)
            ET[(t, h2)][jb] = e
            dead_sp.append(sps[h2])

    def emit_pv(t, h2, jbps, pool=None):
        pr, i0 = BLOCKS[t]
        h = pr * 2 + h2
        if (t, h2) not in PSO:
            pool = pool or psO
            tag = "O" if pool is psO else "S"
            PSO[(t, h2)] = pool.tile([128, IS], F32, tag=tag, name="psO")
        O = PSO[(t, h2)]
        for jbp in jbps:
            va = va_sb[jbp // VA_CHUNK]
            vo = (jbp % VA_CHUNK) * H_PER_CORE * VA_W + h * VA_W
            for nch in range(2):
                nc.tensor.matmul(
                    O[0:128, ds(nch * 512, 512)],
                    va[:, ds(vo, VA_W)],
                    ET[(t, h2)][jbp][:, ds(nch * 512, 512)],
                    start=(jbp == 0), stop=(jbp == JB - 1),
                    skip_group_check=True,
                )

    def emit_norm(t, h2, split=False):
        pr, i0 = BLOCKS[t]
        O = PSO[(t, h2)]
        chunks = [(0, 512), (512, 512)] if split else [(0, IS)]
        for off, w in chunks:
            rr = normp.tile([1, w], F32, tag="rr", name="rr")
            nc.vector.reciprocal_approx_fast(rr[:], O[0:1, ds(off, w)])
            bc = normp.tile([128, w], F32, tag="bc", name="bc")
            nc.gpsimd.partition_broadcast(bc[:], rr[0:1, :])
            if h2 == 1:
                # head1's dims are rows 64-127 of at_sb: write in place
                nc.vector.tensor_mul(
                    at_sb[pr][ds(64, 64), ds(i0 + off, w)],
                    O[64:128, ds(off, w)], bc[64:128, :])
            else:
                nm = normp.tile([128, w], BF16, tag="nm", name="nm")
                nc.vector.tensor_mul(nm[64:128, :], O[64:128, ds(off, w)],
                                     bc[64:128, :])
                nc.sync.dma_start(
                    at_sb[pr][ds(0, 64), ds(i0 + off, w)],
                    nm[64:128, :])
        del ET[(t, h2)]

    def emit_proj(ic, eng="vector"):
        # one i-chunk, full 1024 output columns, through a free psS slot
        Y = psS.tile([128, IS], F32, tag="S", name="Ypj")
        for moch in range(2):
            for hd2 in range(2):
                nc.tensor.matmul(
                    Y[:, ds(moch * 512, 512)],
                    at_sb[hd2][:, ds(ic * 128, 128)],
                    wt_sb[hd2][:, ds(moch * 512, 512)],
                    start=(hd2 == 0), stop=(hd2 == 1),
                    skip_group_check=True,
                )
        ysb = youtp.tile([128, IS], BF16, tag="y")
        if eng == "vector":
            nc.vector.tensor_copy(ysb[:], Y[:])
        else:
            nc.scalar.copy(ysb[:], Y[:])
        nc.sync.dma_start(io["y"][ds(ic * 128, 128), :], ysb[:])

    def emit_fill(mac_ns, half=None):
        # full-array garbage matmuls (K=128, 512 cols): keep the PE's MAC
        # duty above the HAM re-throttle threshold. Target the previous
        # step's h1 S-tile: its exp is (just) done and its psS slot is not
        # re-allocated until the next step's h0 QK, so the pool's slot-reuse
        # dependency keeps everything ordered.
        tgt = dead_sp[-3] if len(dead_sp) >= 3 else dead_sp[0]
        n = max(0, round(mac_ns / FILL_MAC))
        for i in range(n):
            off = 512 * (i % 2 if half is None else half)
            nc.tensor.matmul(tgt[:, ds(off, 512)],
                             kt_sb[0][:, 0:128], kt_sb[0][:, 0:512],
                             start=True, stop=True, skip_group_check=True)

    def emit_proj_inline(unit):
        # a 512-wide projection unit computed in a fill slot: the Y tile is
        # the previous step's dead S-tile (same WAR gate as fills; the slot
        # isn't re-allocated for 1.5 steps — enough for matmuls + copy)
        ic, moch = unit
        tgt = dead_sp[-3]
        for hd2 in range(2):
            nc.tensor.matmul(
                tgt[:, ds(0, 512)],
                at_sb[hd2][:, ds(ic * 128, 128)],
                wt_sb[hd2][:, ds(moch * 512, 512)],
                start=(hd2 == 0), stop=(hd2 == 1),
                skip_group_check=True,
            )
        ysb = youtp.tile([128, 512], BF16, tag="y2", name="ysb2")
        nc.vector.tensor_copy(ysb[:], tgt[:, ds(0, 512)])
        nc.sync.dma_start(io["y"][ds(ic * 128, 128), ds(moch * 512, 512)],
                          ysb[:])

    inline_q = []
    for t in range(len(BLOCKS)):
        for h2 in range(2):
            ET[(t, h2)] = [None] * JB
        for jb in range(JB):
            emit_qk_exp(t, jb)
            pe = QK_PE
            if jb == 1 and t >= 1:
                # previous block's deferred last j-block + its norm (split:
                # the pipelined half-chains free the psO slot ~1us earlier,
                # widening the PSO(t-1,1) alloc margin at step 3)
                emit_pv(t - 1, 0, [15])
                emit_norm(t - 1, 0)
                pe += PV_PE
            if t == 0:
                # no previous-block PV: spread own head0 one j-block per step
                if 1 <= jb < JB - 1:
                    emit_pv(0, 0, [jb - 1])
                    pe += PV_PE
                elif jb == JB - 1:
                    emit_pv(0, 0, [14])
                    pe += PV_PE
            else:
                if jb in PV1_SCHED:
                    jbps = PV1_SCHED[jb]
                    emit_pv(t - 1, 1, jbps)
                    pe += PV_PE * len(jbps)
                    if jb == 7:
                        emit_norm(t - 1, 1)
                if jb in PV0_SCHED:
                    jbps = PV0_SCHED[jb]
                    emit_pv(t, 0, jbps)
                    pe += PV_PE * len(jbps)
            # in block 3, i-chunks 0-7 are fully normalized: spend fill
            # slots on real projection units instead of garbage
            inline = t == 3 and jb in (0, 1, 2, 7, 8, 9, 14, 15) and inline_q
            if inline:
                emit_proj_inline(inline_q.pop(0))
                pe += PV_PE
            # fill-light steps stall on the fill gate (their [-3] S-tile's
            # exp is still in flight when the PE runs ahead); extra fills
            # behind that gate would only delay the next step's QK
            budget = PE_TARGET - pe
            if pe < 1500:
                budget = min(budget, (1 if inline else 3) * FILL_MAC)
            emit_fill(budget, half=1 if inline else None)

    # tail: block 3's head-1 PV accumulates in a freed psS slot (j-blocks
    # 0..13 are ungated and bridge the last exps' retirement), the deferred
    # head0 j-block + norms slot in behind, then the entire projection
    # pipelines through the remaining psS slots
    emit_pv(3, 1, list(range(JB - 2)), pool=psS)
    emit_pv(3, 0, [15])
    emit_norm(3, 0)
    emit_pv(3, 1, [14, 15])
    emit_norm(3, 1, split=True)
    for ic in range(0, 16):
        emit_proj(ic, eng=("scalar" if ic % 2 else "vector"))


_CACHED_NC = None


def _build():
    global _CACHED_NC
    if _CACHED_NC is not None:
        return _CACHED_NC
    nc = bacc.Bacc("TRN2", target_bir_lowering=False, debug=False,
                   num_devices=N_CORES)
    va_cols = JB // VA_CHUNK * H_PER_CORE * VA_W
    io = {
        "qt": nc.dram_tensor("qt", [2, 128, S], BF16,
                             kind="ExternalInput").ap(),
        "kt": nc.dram_tensor("kt", [2, 128, S], BF16,
                             kind="ExternalInput").ap(),
        "va": nc.dram_tensor("va", [VA_CHUNK, 128, va_cols], BF16,
                             kind="ExternalInput").ap(),
        "wt": nc.dram_tensor("wt", [2, 128, D_MODEL], BF16,
                             kind="ExternalInput").ap(),
        "y": nc.dram_tensor("y", [S, D_MODEL], BF16,
                            kind="ExternalOutput").ap(),
        "warm": nc.dram_tensor("warm", [1, 128], F32,
                               kind="ExternalOutput").ap(),
        "wexp": nc.dram_tensor("wexp", [1, 128], F32,
                               kind="ExternalOutput").ap(),
    }
    with tile.TileContext(nc) as tc:
        with ExitStack() as ctx:
            _trace(ctx, tc, io)
    nc.compile()
    _CACHED_NC = nc
    return nc


def _core_inputs(q, k, v, W, b, core):
    bb, g = divmod(core, 4)
    hd0 = g * H_PER_CORE * D_K  # 256 per group
    ncol = H_PER_CORE * D_K
    bf = ml_dtypes.bfloat16

    qt = np.ascontiguousarray(q[bb, :, hd0:hd0 + ncol].T).reshape(2, 128, S)
    kt = np.ascontiguousarray(k[bb, :, hd0:hd0 + ncol].T).reshape(2, 128, S)
    v_sl = v[bb, :, hd0:hd0 + ncol].reshape(S, H_PER_CORE, D_K)
    va = np.concatenate(
        [np.ones((S, H_PER_CORE, 1), np.float32),
         np.zeros((S, H_PER_CORE, 63), np.float32), v_sl], axis=2
    ).reshape(JB, 128, H_PER_CORE * VA_W).transpose(1, 0, 2).reshape(
        128, JB * H_PER_CORE * VA_W)
    va = va.reshape(128, VA_CHUNK, JB // VA_CHUNK * H_PER_CORE * VA_W)
    va = np.ascontiguousarray(va.transpose(1, 0, 2))
    wt = np.ascontiguousarray(W[:, hd0:hd0 + ncol].T).reshape(2, 128, D_MODEL)
    return {
        "qt": qt.astype(bf),
        "kt": kt.astype(bf),
        "va": va.astype(bf),
        "wt": wt.astype(bf),
    }


def run(inputs, trace=False, trace_kwargs=None):
    from concourse.bass_utils import run_bass_kernel_spmd

    q = np.asarray(inputs["q"], np.float32)
    k = np.asarray(inputs["k"], np.float32)
    v = np.asarray(inputs["v"], np.float32)
    W = np.asarray(inputs["W"], np.float32)
    b = np.asarray(inputs["b"], np.float32)

    nc = _build()
    in_maps = [_core_inputs(q, k, v, W, b, c) for c in range(N_CORES)]
    res = run_bass_kernel_spmd(nc, in_maps, core_ids=list(range(N_CORES)),
                               trace=trace, **(trace_kwargs or {}))
    out = np.empty((B, S, D_MODEL), np.float32)
    for bb in range(B):
        acc = res.results[bb * 4 + 0]["y"].astype(np.float32)
        for g in range(1, 4):
            acc = acc + res.results[bb * 4 + g]["y"].astype(np.float32)
        out[bb] = acc + b[None, :]
    return out, res


def kernel(**inputs):
    out, _ = run(inputs)
    return out
